# revision 36
# baseline (speedup 1.0000x reference)
"""CommNet actor kernel for Trainium2 (Bass/Tile), 8-core data-parallel.

Math (per sample, A=32 agents, D=128 obs, H=64 hidden, NA=16 actions):
    h   = tanh(obs @ enc_w + enc_b)
    2 rounds of:  messages = h @ comm_w + comm_b
                  received = (sum_agents(messages) - messages) / (A-1)
                  h = tanh([h, received] @ upd_w + upd_b)
    out = tanh(h @ dec_w1 + dec_b1) @ dec_w2 + dec_b2

The round is folded on the host into  h' = tanh(h @ W1 + s @ W2 + b)  where
s = sum_agents(h), W1 = U_top - comm_w @ U_bot / (A-1), W2 = comm_w @ U_bot / (A-1),
b = comm_b @ U_bot + upd_b   (U_top/U_bot = upd_w[:H], upd_w[H:]).

Device layout: feature-major activations [feat, tok]. Each "unit" is 2048
tokens; the first 1024 tokens (T0) live on SBUF/PSUM partitions 0:64, the
second 1024 (T1) on partitions 64:128. All matmuls run in bf16 (1 cycle/col
moving operand; fp32/f32r move at 2 cyc/col). The 64x64 round weights are
packed as block-diagonal kron(I2, W) so one full-array matmul covers both
token halves. The encoder (128-contraction, 64 out) uses two concurrent
tile_position col-tiles, (0,0) for T0 and (0,64) for T1, so the two halves
stream through the PE at the same time. dec2 (32 out rows) uses four
concurrent 32-partition col-tiles, which also spreads its output over all
128 partitions so the PSUM->SBUF drain copy runs full-lane on the DVE.
tanh/reduce process both halves in single [128, 1024] instructions.

obs is pre-transposed on the host into the exact feature-major DMA layout, so
all HBM traffic is contiguous; the output is stored bf16 in DMA walk order and
decoded (plus the final fp32 dec_b2 add) on the host.
"""

import numpy as np
from contextlib import ExitStack

import concourse.bass as bass
import concourse.bacc as bacc
import concourse.tile as tile
from concourse import mybir
from concourse.bass_utils import run_bass_kernel_spmd

# Problem constants
B, A, D, H, NA = 16384, 32, 128, 64, 16
R = 2
NCORES = 8
S_CORE = B // NCORES          # 2048 samples per core
TOK = S_CORE * A              # 65536 tokens per core
HALF_TOK = 1024               # tokens per half-unit (32 samples)
UNIT_TOK = 2 * HALF_TOK       # 2048 tokens per unit
NU = TOK // UNIT_TOK          # 32 units per core
SAMP_HALF = HALF_TOK // A     # 32 samples per half-unit
STRIP = 4                     # dec2 col-tiles
STOK = HALF_TOK // STRIP      # 256 tokens per dec2 strip
FP = mybir.dt.float32
FR = mybir.dt.float32r
BF = mybir.dt.bfloat16
TANH = mybir.ActivationFunctionType.Tanh


def _f(ap):
    return ap.bitcast(FP)


# wpack16 (bf16) column layout: all matmul weights
_C_ENC = 0                    # enc_w                 [128, 64]
_C_W1 = (64, 192)             # kron(I2, W1_r)        [128, 128] per round
_C_W2 = (320, 448)            # kron(I2, W2_r)        [128, 128] per round
_C_D1 = 576                   # kron(I2, dec_w1)      [128, 128]
_C_D2 = 704                   # kron(I2, dec_w2)      [128, 32]
NW16 = 736
# wpack (fp32) column layout: bias columns enc, r0, r1, dec1 (stacked [b; b])
NW = 4


def build_body(ctx, tc, obs_t, wpack, out, n_units):
    nc = tc.nc
    wpool = ctx.enter_context(tc.tile_pool(name="w", bufs=1))
    obs_pool = ctx.enter_context(tc.tile_pool(name="obs", bufs=3))
    h_pool = ctx.enter_context(tc.tile_pool(name="h", bufs=24))
    s_pool = ctx.enter_context(tc.tile_pool(name="s", bufs=16))
    osb_pool = ctx.enter_context(tc.tile_pool(name="osb", bufs=3))
    ps_pool = ctx.enter_context(tc.tile_pool(name="ps", bufs=3, space="PSUM"))
    po_pool = ctx.enter_context(tc.tile_pool(name="po", bufs=2, space="PSUM"))

    wpack, wpack16 = wpack
    w = wpool.tile([D, NW], FR)
    w16 = wpool.tile([D, NW16], BF)

    def emit_wload():
        nc.sync.dma_start(out=w16[:], in_=wpack16)
        nc.sync.dma_start(out=w[:], in_=wpack)

    def emit_warm():
        # Prime the ACT table (~2.7us TABLE_LOAD+DRAIN) during the DMA ramp so
        # the first real tanh doesn't pay it: a 1-element dummy with no DMA
        # deps. Emitted after the first obs loads — DMA issue shares the
        # Scalar sequencer, so putting this first would delay those issues.
        warm = wpool.tile([128, 1], FP)
        nc.vector.memset(warm[:], 0.0)
        nc.scalar.activation(warm[:], warm[:], TANH)
        # Un-throttle the PE during the DMA ramp: the HAM clock gate keeps the
        # array at 1.2 GHz until it has been busy for a full ~3.4us window, so
        # feed it garbage matmuls (into a scratch psum tile) before the first
        # obs tile lands. Otherwise groups 0-1 run their matmuls at half clock.
        wmm = wpool.tile([128, 512], BF)
        nc.vector.memset(wmm[:], 0.0)
        ps_w = po_pool.tile([128, STOK], FP, tag="po")
        for _ in range(16):
            nc.tensor.matmul(ps_w[0:32, :], lhsT=wmm[:, 0:32],
                             rhs=wmm[:, 0:STOK], tile_position=(0, 0),
                             skip_group_check=True)

    w_enc = w16[:, _C_ENC : _C_ENC + 64]
    w1 = [w16[:, _C_W1[r] : _C_W1[r] + 128] for r in range(R)]
    w2 = [w16[:, _C_W2[r] : _C_W2[r] + 128] for r in range(R)]
    w_d1 = w16[:, _C_D1 : _C_D1 + 128]
    w_d2 = w16[:, _C_D2 : _C_D2 + 32]
    b_enc = _f(w[:, 0:1])
    b_r = [_f(w[:, 1 + r : 2 + r]) for r in range(R)]
    b_d1 = _f(w[:, 3:4])

    c0 = slice(0, 512)
    c1 = slice(512, 1024)

    def emit_loads(gi, gsize):
        # one dma_start per 4-unit group with 16KB contiguous per partition
        # line: DMA issue costs ~610ns of serial sequencer time and each
        # descriptor has ~300-400ns of fixed overhead, so big lines win
        ob = obs_pool.tile([D, 4, 2, HALF_TOK], BF, tag="obs")
        nc.sync.dma_start(out=ob[:, 0:gsize], in_=obs_t[gi, :, 0:gsize])
        return [(ob[:, k, 0, :], ob[:, k, 1, :]) for k in range(gsize)]

    def emit_enc_mms(obs0, obs1):
        # two concurrent col-tiles: T0 -> psum partitions 0:64 via array cols
        # 0:63, T1 -> partitions 64:128 via cols 64:127
        ps_e = ps_pool.tile([128, HALF_TOK], FP, tag="ps")
        for cs in (c0, c1):
            nc.tensor.matmul(ps_e[0:64, cs], lhsT=w_enc, rhs=obs0[:, cs],
                             tile_position=(0, 0), skip_group_check=True)
            nc.tensor.matmul(ps_e[64:128, cs], lhsT=w_enc, rhs=obs1[:, cs],
                             tile_position=(0, 64), skip_group_check=True)
        return ps_e

    def emit_tanh(ps, bias, split=False):
        hh = h_pool.tile([128, HALF_TOK], BF, tag="h")
        if split:  # first unit: let the first half-tile tanh start sooner
            nc.scalar.activation(hh[:, c0], ps[:, c0], TANH, bias=bias)
            nc.scalar.activation(hh[:, c1], ps[:, c1], TANH, bias=bias)
        else:
            nc.scalar.activation(hh[:], ps[:], TANH, bias=bias)
        return hh

    def emit_reduce(hh):
        s = s_pool.tile([128, SAMP_HALF], hh.dtype, tag="s")
        with nc.allow_low_precision(
            reason="bf16 agent-sum feeds a bf16 matmul; fp32 internal accum"
        ):
            nc.vector.reduce_sum(
                out=s[:],
                in_=hh.rearrange("p (g a) -> p g a", a=A),
                axis=mybir.AxisListType.X,
            )
        return s

    def emit_round_mms(r, hh, s):
        ns = SAMP_HALF // 2  # samples per 512-token column block
        ps_r = ps_pool.tile([128, HALF_TOK], FP, tag="ps")
        for cs in (c0, c1):
            nc.tensor.matmul(ps_r[:, cs], lhsT=w1[r], rhs=hh[:, cs],
                             start=True, stop=False, skip_group_check=True)
        for b, cs in ((0, c0), (1, c1)):
            sb = s[:, b * ns : (b + 1) * ns].unsqueeze(2).broadcast_to(
                [128, ns, A]
            )
            nc.tensor.matmul(ps_r[:, cs], lhsT=w2[r], rhs=sb,
                             start=False, stop=True, skip_group_check=True)
        return ps_r

    def emit_dec1_mms(hh):
        ps_d = ps_pool.tile([128, HALF_TOK], FP, tag="ps")
        for cs in (c0, c1):
            nc.tensor.matmul(ps_d[:, cs], lhsT=w_d1, rhs=hh[:, cs],
                             skip_group_check=True)
        return ps_d

    def emit_dec2_mms(pre):
        # four concurrent 32-partition col-tiles; strip k covers token block
        # k*256:(k+1)*256 and lands on psum partitions 32k:32k+32
        po = po_pool.tile([128, STOK], FP, tag="po")
        for k in range(STRIP):
            nc.tensor.matmul(po[32 * k : 32 * k + 32, :], lhsT=w_d2,
                             rhs=pre[:, k * STOK : (k + 1) * STOK],
                             tile_position=(0, 32 * k), skip_group_check=True)
        return po

    def emit_dec2_cast(osb, k, po):
        nc.vector.tensor_copy(osb[:, k, :], po[:])

    # Units are emitted in interleaved groups of four (software pipelining
    # at the emission level): one unit's matmuls fill the PE gaps left by
    # another unit's tanh/reduce stages. Without this the PE idles ~1.3us at
    # every stage boundary and the HAM clock-gate re-throttles it to 1.2 GHz.
    groups = []
    u0 = 0
    while u0 < n_units:
        g = 4 if n_units - u0 >= 4 else n_units - u0
        groups.append(list(range(u0, u0 + g)))
        u0 += g
    for gi, grp in enumerate(groups):
        obs = emit_loads(gi, len(grp))
        if gi == 0:
            emit_wload()
            emit_warm()
        ps = [emit_enc_mms(*o) for o in obs]
        hs = [emit_tanh(p, b_enc) for p in ps]
        for r in range(R):
            ss = [emit_reduce(hh) for hh in hs]
            ps = [emit_round_mms(r, hh, s) for hh, s in zip(hs, ss)]
            hs = [emit_tanh(p, b_r[r]) for p in ps]
        ps = [emit_dec1_mms(hh) for hh in hs]
        osb = osb_pool.tile([128, len(grp), STOK], BF, tag="osb")
        for k, p in enumerate(ps):
            pre = emit_tanh(p, b_d1)
            po = emit_dec2_mms(pre)
            emit_dec2_cast(osb, k, po)
            if k % 2 == 1:  # flush per unit-pair so the tail drain is short
                nc.sync.dma_start(
                    out=out[:, grp[k - 1] : grp[k] + 1, :],
                    in_=osb[:, k - 1 : k + 1, :],
                )
        if len(grp) % 2 == 1:
            k = len(grp) - 1
            nc.sync.dma_start(
                out=out[:, grp[k] : grp[k] + 1, :], in_=osb[:, k : k + 1, :]
            )


def build_nc(n_units=NU):
    nc = bacc.Bacc(None, target_bir_lowering=False, debug=False)
    n_groups = (n_units + 3) // 4
    obs_t = nc.declare_dram_parameter(
        "obs_t", [n_groups, D, 4, 2, HALF_TOK], BF, isOutput=False
    )
    wpack = nc.declare_dram_parameter("wpack", [D, NW], FR, isOutput=False)
    wpack16 = nc.declare_dram_parameter("wpack16", [D, NW16], BF, isOutput=False)
    out = nc.declare_dram_parameter(
        "out", [128, n_units, STOK], BF, isOutput=True
    )
    with tile.TileContext(nc) as tc:
        with ExitStack() as ctx:
            build_body(ctx, tc, obs_t[:], (wpack[:], wpack16[:]), out[:], n_units)
    nc.compile()
    return nc


def fold_weights(enc_w, enc_b, comm_w, comm_b, upd_w, upd_b, dec_w1, dec_b1, dec_w2):
    """Host-side algebraic fold + packing into the wpack tensors (float64 math)."""
    import ml_dtypes

    f8 = np.float64
    denom = f8(max(A - 1, 1))
    wpack = np.zeros((D, NW), np.float32)
    wpack16 = np.zeros((D, NW16), np.float32)

    def bd(Wm):  # kron(I2, W) for [64, x] -> [128, 2x]
        Wm = np.asarray(Wm, np.float32)
        k, m = Wm.shape
        o = np.zeros((2 * k, 2 * m), np.float32)
        o[:k, :m] = Wm
        o[k:, m:] = Wm
        return o

    wpack16[:, _C_ENC : _C_ENC + 64] = np.asarray(enc_w, np.float32)
    for r in range(R):
        C = np.asarray(comm_w[r], f8)
        Ut = np.asarray(upd_w[r][:H], f8)
        Ub = np.asarray(upd_w[r][H:], f8)
        G = C @ Ub / denom
        W1 = (Ut - G).astype(np.float32)
        W2 = G.astype(np.float32)
        br = (np.asarray(comm_b[r], f8) @ Ub + np.asarray(upd_b[r], f8)).astype(
            np.float32
        )
        wpack16[:, _C_W1[r] : _C_W1[r] + 128] = bd(W1)
        wpack16[:, _C_W2[r] : _C_W2[r] + 128] = bd(W2)
        wpack[0:64, 1 + r] = br
        wpack[64:128, 1 + r] = br
    wpack16[:, _C_D1 : _C_D1 + 128] = bd(dec_w1)
    wpack16[:, _C_D2 : _C_D2 + 32] = bd(dec_w2)
    be = np.asarray(enc_b, np.float32)
    wpack[0:64, 0] = be
    wpack[64:128, 0] = be
    bd1 = np.asarray(dec_b1, np.float32)
    wpack[0:64, 3] = bd1
    wpack[64:128, 3] = bd1
    return wpack, wpack16.astype(ml_dtypes.bfloat16)


def prep_obs(obs):
    """[B, A, D] -> [NCORES, NU//4, D, 4, 2, HALF_TOK] feature-major bf16."""
    import ml_dtypes

    obs6 = np.asarray(obs, np.float32).reshape(NCORES, NU // 4, 4, 2, HALF_TOK, D)
    return np.ascontiguousarray(
        obs6.transpose(0, 1, 5, 2, 3, 4).astype(ml_dtypes.bfloat16)
    )


_NC_CACHE = {}


def _get_nc(n_units=NU):
    if n_units not in _NC_CACHE:
        _NC_CACHE[n_units] = build_nc(n_units)
    return _NC_CACHE[n_units]


def kernel(
    obs,
    enc_w,
    enc_b,
    comm_w,
    comm_b,
    upd_w,
    upd_b,
    dec_w1,
    dec_b1,
    dec_w2,
    dec_b2,
    _trace=False,
    _trace_kwargs=None,
):
    wpack, wpack16 = fold_weights(
        enc_w, enc_b, comm_w, comm_b, upd_w, upd_b, dec_w1, dec_b1, dec_w2
    )
    obs_t = prep_obs(obs)
    nc = _get_nc()
    in_maps = [
        {"obs_t": obs_t[i], "wpack": wpack, "wpack16": wpack16}
        for i in range(NCORES)
    ]
    res = run_bass_kernel_spmd(
        nc,
        in_maps,
        core_ids=list(range(NCORES)),
        trace=_trace,
        **(_trace_kwargs or {}),
    )
    outs = np.stack([res.results[i]["out"] for i in range(NCORES)])
    # device order: out[32*strip + 16*half + e, u, c]; token within the half
    # is strip*256 + c
    outs = np.asarray(outs, np.float32).reshape(NCORES, STRIP, 2, NA, NU, STOK)
    outs = outs.transpose(0, 4, 2, 1, 5, 3)  # -> [core, u, half, strip, c, e]
    logits = outs.reshape(B, A, NA) + np.asarray(dec_b2, np.float32)[None, None, :]
    if _trace:
        return logits.astype(np.float32), res
    return logits.astype(np.float32)


# revision 38
# speedup vs baseline: 1.0066x; 1.0066x over previous
"""CommNet actor kernel for Trainium2 (Bass/Tile), 8-core data-parallel.

Math (per sample, A=32 agents, D=128 obs, H=64 hidden, NA=16 actions):
    h   = tanh(obs @ enc_w + enc_b)
    2 rounds of:  messages = h @ comm_w + comm_b
                  received = (sum_agents(messages) - messages) / (A-1)
                  h = tanh([h, received] @ upd_w + upd_b)
    out = tanh(h @ dec_w1 + dec_b1) @ dec_w2 + dec_b2

The round is folded on the host into  h' = tanh(h @ W1 + s @ W2 + b)  where
s = sum_agents(h), W1 = U_top - comm_w @ U_bot / (A-1), W2 = comm_w @ U_bot / (A-1),
b = comm_b @ U_bot + upd_b   (U_top/U_bot = upd_w[:H], upd_w[H:]).

Device layout: feature-major activations [feat, tok]. Each "unit" is 2048
tokens; the first 1024 tokens (T0) live on SBUF/PSUM partitions 0:64, the
second 1024 (T1) on partitions 64:128. All matmuls run in bf16 (1 cycle/col
moving operand; fp32/f32r move at 2 cyc/col). The 64x64 round weights are
packed as block-diagonal kron(I2, W) so one full-array matmul covers both
token halves. The encoder (128-contraction, 64 out) uses two concurrent
tile_position col-tiles, (0,0) for T0 and (0,64) for T1, so the two halves
stream through the PE at the same time. dec2 (32 out rows) uses four
concurrent 32-partition col-tiles, which also spreads its output over all
128 partitions so the PSUM->SBUF drain copy runs full-lane on the DVE.
tanh/reduce process both halves in single [128, 1024] instructions.

obs is pre-transposed on the host into the exact feature-major DMA layout, so
all HBM traffic is contiguous; the output is stored bf16 in DMA walk order and
decoded (plus the final fp32 dec_b2 add) on the host.
"""

import numpy as np
from contextlib import ExitStack

import concourse.bass as bass
import concourse.bacc as bacc
import concourse.tile as tile
from concourse import mybir
from concourse.bass_utils import run_bass_kernel_spmd

# Problem constants
B, A, D, H, NA = 16384, 32, 128, 64, 16
R = 2
NCORES = 8
S_CORE = B // NCORES          # 2048 samples per core
TOK = S_CORE * A              # 65536 tokens per core
HALF_TOK = 1024               # tokens per half-unit (32 samples)
UNIT_TOK = 2 * HALF_TOK       # 2048 tokens per unit
NU = TOK // UNIT_TOK          # 32 units per core
SAMP_HALF = HALF_TOK // A     # 32 samples per half-unit
STRIP = 4                     # dec2 col-tiles
STOK = HALF_TOK // STRIP      # 256 tokens per dec2 strip
FP = mybir.dt.float32
FR = mybir.dt.float32r
BF = mybir.dt.bfloat16
TANH = mybir.ActivationFunctionType.Tanh


def _f(ap):
    return ap.bitcast(FP)


# wpack16 (bf16) column layout: all matmul weights
_C_ENC = 0                    # enc_w                 [128, 64]
_C_W1 = (64, 192)             # kron(I2, W1_r)        [128, 128] per round
_C_W2 = (320, 448)            # kron(I2, W2_r)        [128, 128] per round
_C_D1 = 576                   # kron(I2, dec_w1)      [128, 128]
_C_D2 = 704                   # kron(I2, dec_w2)      [128, 32]
NW16 = 736
# wpack (fp32) column layout: bias columns enc, r0, r1, dec1 (stacked [b; b])
NW = 4


def build_body(ctx, tc, obs_t, wpack, out, n_units):
    nc = tc.nc
    wpool = ctx.enter_context(tc.tile_pool(name="w", bufs=1))
    obs_pool = ctx.enter_context(tc.tile_pool(name="obs", bufs=12))
    h_pool = ctx.enter_context(tc.tile_pool(name="h", bufs=24))
    s_pool = ctx.enter_context(tc.tile_pool(name="s", bufs=16))
    osb_pool = ctx.enter_context(tc.tile_pool(name="osb", bufs=3))
    ps_pool = ctx.enter_context(tc.tile_pool(name="ps", bufs=3, space="PSUM"))
    po_pool = ctx.enter_context(tc.tile_pool(name="po", bufs=2, space="PSUM"))

    wpack, wpack16 = wpack
    w = wpool.tile([D, NW], FR)
    w16 = wpool.tile([D, NW16], BF)

    def emit_wload():
        nc.sync.dma_start(out=w16[:], in_=wpack16)
        nc.sync.dma_start(out=w[:], in_=wpack)

    def emit_warm():
        # Prime the ACT table (~2.7us TABLE_LOAD+DRAIN) during the DMA ramp so
        # the first real tanh doesn't pay it: a 1-element dummy with no DMA
        # deps. Emitted after the first obs loads — DMA issue shares the
        # Scalar sequencer, so putting this first would delay those issues.
        warm = wpool.tile([128, 1], FP)
        nc.vector.memset(warm[:], 0.0)
        nc.scalar.activation(warm[:], warm[:], TANH)
        # Un-throttle the PE during the DMA ramp: the HAM clock gate keeps the
        # array at 1.2 GHz until it has been busy for a full ~3.4us window, so
        # feed it garbage matmuls (into a scratch psum tile) before the first
        # obs tile lands. Otherwise groups 0-1 run their matmuls at half clock.
        wmm = wpool.tile([128, 512], BF)
        nc.vector.memset(wmm[:], 0.0)
        ps_w = po_pool.tile([128, STOK], FP, tag="po")
        for _ in range(16):
            nc.tensor.matmul(ps_w[0:32, :], lhsT=wmm[:, 0:32],
                             rhs=wmm[:, 0:STOK], tile_position=(0, 0),
                             skip_group_check=True)

    w_enc = w16[:, _C_ENC : _C_ENC + 64]
    w1 = [w16[:, _C_W1[r] : _C_W1[r] + 128] for r in range(R)]
    w2 = [w16[:, _C_W2[r] : _C_W2[r] + 128] for r in range(R)]
    w_d1 = w16[:, _C_D1 : _C_D1 + 128]
    w_d2 = w16[:, _C_D2 : _C_D2 + 32]
    b_enc = _f(w[:, 0:1])
    b_r = [_f(w[:, 1 + r : 2 + r]) for r in range(R)]
    b_d1 = _f(w[:, 3:4])

    c0 = slice(0, 512)
    c1 = slice(512, 1024)

    def emit_loads(gi, gsize):
        # one dma_start per unit (4KB contiguous per partition line): bigger
        # per-group transfers would amortize descriptor overhead better, but
        # tile-granular deps then stall the first unit on the whole transfer
        obs = []
        for k in range(gsize):
            ob = obs_pool.tile([D, 2, HALF_TOK], BF, tag="obs")
            nc.sync.dma_start(out=ob[:], in_=obs_t[gi, :, k])
            obs.append((ob[:, 0, :], ob[:, 1, :]))
        return obs

    def emit_enc_mms(obs0, obs1):
        # two concurrent col-tiles: T0 -> psum partitions 0:64 via array cols
        # 0:63, T1 -> partitions 64:128 via cols 64:127
        ps_e = ps_pool.tile([128, HALF_TOK], FP, tag="ps")
        for cs in (c0, c1):
            nc.tensor.matmul(ps_e[0:64, cs], lhsT=w_enc, rhs=obs0[:, cs],
                             tile_position=(0, 0), skip_group_check=True)
            nc.tensor.matmul(ps_e[64:128, cs], lhsT=w_enc, rhs=obs1[:, cs],
                             tile_position=(0, 64), skip_group_check=True)
        return ps_e

    def emit_tanh(ps, bias, split=False):
        hh = h_pool.tile([128, HALF_TOK], BF, tag="h")
        if split:  # first unit: let the first half-tile tanh start sooner
            nc.scalar.activation(hh[:, c0], ps[:, c0], TANH, bias=bias)
            nc.scalar.activation(hh[:, c1], ps[:, c1], TANH, bias=bias)
        else:
            nc.scalar.activation(hh[:], ps[:], TANH, bias=bias)
        return hh

    def emit_reduce(hh):
        s = s_pool.tile([128, SAMP_HALF], hh.dtype, tag="s")
        with nc.allow_low_precision(
            reason="bf16 agent-sum feeds a bf16 matmul; fp32 internal accum"
        ):
            nc.vector.reduce_sum(
                out=s[:],
                in_=hh.rearrange("p (g a) -> p g a", a=A),
                axis=mybir.AxisListType.X,
            )
        return s

    def emit_round_mms(r, hh, s):
        ns = SAMP_HALF // 2  # samples per 512-token column block
        ps_r = ps_pool.tile([128, HALF_TOK], FP, tag="ps")
        for cs in (c0, c1):
            nc.tensor.matmul(ps_r[:, cs], lhsT=w1[r], rhs=hh[:, cs],
                             start=True, stop=False, skip_group_check=True)
        for b, cs in ((0, c0), (1, c1)):
            sb = s[:, b * ns : (b + 1) * ns].unsqueeze(2).broadcast_to(
                [128, ns, A]
            )
            nc.tensor.matmul(ps_r[:, cs], lhsT=w2[r], rhs=sb,
                             start=False, stop=True, skip_group_check=True)
        return ps_r

    def emit_dec1_mms(hh):
        ps_d = ps_pool.tile([128, HALF_TOK], FP, tag="ps")
        for cs in (c0, c1):
            nc.tensor.matmul(ps_d[:, cs], lhsT=w_d1, rhs=hh[:, cs],
                             skip_group_check=True)
        return ps_d

    def emit_dec2_mms(pre):
        # four concurrent 32-partition col-tiles; strip k covers token block
        # k*256:(k+1)*256 and lands on psum partitions 32k:32k+32
        po = po_pool.tile([128, STOK], FP, tag="po")
        for k in range(STRIP):
            nc.tensor.matmul(po[32 * k : 32 * k + 32, :], lhsT=w_d2,
                             rhs=pre[:, k * STOK : (k + 1) * STOK],
                             tile_position=(0, 32 * k), skip_group_check=True)
        return po

    def emit_dec2_cast(osb, k, po):
        nc.vector.tensor_copy(osb[:, k, :], po[:])

    # Units are emitted in interleaved groups of four (software pipelining
    # at the emission level): one unit's matmuls fill the PE gaps left by
    # another unit's tanh/reduce stages. Without this the PE idles ~1.3us at
    # every stage boundary and the HAM clock-gate re-throttles it to 1.2 GHz.
    groups = []
    u0 = 0
    while u0 < n_units:
        g = 4 if n_units - u0 >= 4 else n_units - u0
        groups.append(list(range(u0, u0 + g)))
        u0 += g
    for gi, grp in enumerate(groups):
        obs = emit_loads(gi, len(grp))
        if gi == 0:
            emit_wload()
            emit_warm()
        ps = [emit_enc_mms(*o) for o in obs]
        hs = [emit_tanh(p, b_enc) for p in ps]
        for r in range(R):
            ss = [emit_reduce(hh) for hh in hs]
            ps = [emit_round_mms(r, hh, s) for hh, s in zip(hs, ss)]
            hs = [emit_tanh(p, b_r[r]) for p in ps]
        ps = [emit_dec1_mms(hh) for hh in hs]
        osb = osb_pool.tile([128, len(grp), STOK], BF, tag="osb")
        for k, p in enumerate(ps):
            pre = emit_tanh(p, b_d1)
            po = emit_dec2_mms(pre)
            emit_dec2_cast(osb, k, po)
            if k % 2 == 1:  # flush per unit-pair so the tail drain is short
                nc.sync.dma_start(
                    out=out[:, grp[k - 1] : grp[k] + 1, :],
                    in_=osb[:, k - 1 : k + 1, :],
                )
        if len(grp) % 2 == 1:
            k = len(grp) - 1
            nc.sync.dma_start(
                out=out[:, grp[k] : grp[k] + 1, :], in_=osb[:, k : k + 1, :]
            )


def build_nc(n_units=NU):
    nc = bacc.Bacc(None, target_bir_lowering=False, debug=False)
    n_groups = (n_units + 3) // 4
    obs_t = nc.declare_dram_parameter(
        "obs_t", [n_groups, D, 4, 2, HALF_TOK], BF, isOutput=False
    )
    wpack = nc.declare_dram_parameter("wpack", [D, NW], FR, isOutput=False)
    wpack16 = nc.declare_dram_parameter("wpack16", [D, NW16], BF, isOutput=False)
    out = nc.declare_dram_parameter(
        "out", [128, n_units, STOK], BF, isOutput=True
    )
    with tile.TileContext(nc) as tc:
        with ExitStack() as ctx:
            build_body(ctx, tc, obs_t[:], (wpack[:], wpack16[:]), out[:], n_units)
    nc.compile()
    return nc


def fold_weights(enc_w, enc_b, comm_w, comm_b, upd_w, upd_b, dec_w1, dec_b1, dec_w2):
    """Host-side algebraic fold + packing into the wpack tensors (float64 math)."""
    import ml_dtypes

    f8 = np.float64
    denom = f8(max(A - 1, 1))
    wpack = np.zeros((D, NW), np.float32)
    wpack16 = np.zeros((D, NW16), np.float32)

    def bd(Wm):  # kron(I2, W) for [64, x] -> [128, 2x]
        Wm = np.asarray(Wm, np.float32)
        k, m = Wm.shape
        o = np.zeros((2 * k, 2 * m), np.float32)
        o[:k, :m] = Wm
        o[k:, m:] = Wm
        return o

    wpack16[:, _C_ENC : _C_ENC + 64] = np.asarray(enc_w, np.float32)
    for r in range(R):
        C = np.asarray(comm_w[r], f8)
        Ut = np.asarray(upd_w[r][:H], f8)
        Ub = np.asarray(upd_w[r][H:], f8)
        G = C @ Ub / denom
        W1 = (Ut - G).astype(np.float32)
        W2 = G.astype(np.float32)
        br = (np.asarray(comm_b[r], f8) @ Ub + np.asarray(upd_b[r], f8)).astype(
            np.float32
        )
        wpack16[:, _C_W1[r] : _C_W1[r] + 128] = bd(W1)
        wpack16[:, _C_W2[r] : _C_W2[r] + 128] = bd(W2)
        wpack[0:64, 1 + r] = br
        wpack[64:128, 1 + r] = br
    wpack16[:, _C_D1 : _C_D1 + 128] = bd(dec_w1)
    wpack16[:, _C_D2 : _C_D2 + 32] = bd(dec_w2)
    be = np.asarray(enc_b, np.float32)
    wpack[0:64, 0] = be
    wpack[64:128, 0] = be
    bd1 = np.asarray(dec_b1, np.float32)
    wpack[0:64, 3] = bd1
    wpack[64:128, 3] = bd1
    return wpack, wpack16.astype(ml_dtypes.bfloat16)


def prep_obs(obs):
    """[B, A, D] -> [NCORES, NU//4, D, 4, 2, HALF_TOK] feature-major bf16."""
    import ml_dtypes

    obs6 = np.asarray(obs, np.float32).reshape(NCORES, NU // 4, 4, 2, HALF_TOK, D)
    return np.ascontiguousarray(
        obs6.transpose(0, 1, 5, 2, 3, 4).astype(ml_dtypes.bfloat16)
    )


_NC_CACHE = {}


def _get_nc(n_units=NU):
    if n_units not in _NC_CACHE:
        _NC_CACHE[n_units] = build_nc(n_units)
    return _NC_CACHE[n_units]


def kernel(
    obs,
    enc_w,
    enc_b,
    comm_w,
    comm_b,
    upd_w,
    upd_b,
    dec_w1,
    dec_b1,
    dec_w2,
    dec_b2,
    _trace=False,
    _trace_kwargs=None,
):
    wpack, wpack16 = fold_weights(
        enc_w, enc_b, comm_w, comm_b, upd_w, upd_b, dec_w1, dec_b1, dec_w2
    )
    obs_t = prep_obs(obs)
    nc = _get_nc()
    in_maps = [
        {"obs_t": obs_t[i], "wpack": wpack, "wpack16": wpack16}
        for i in range(NCORES)
    ]
    res = run_bass_kernel_spmd(
        nc,
        in_maps,
        core_ids=list(range(NCORES)),
        trace=_trace,
        **(_trace_kwargs or {}),
    )
    outs = np.stack([res.results[i]["out"] for i in range(NCORES)])
    # device order: out[32*strip + 16*half + e, u, c]; token within the half
    # is strip*256 + c
    outs = np.asarray(outs, np.float32).reshape(NCORES, STRIP, 2, NA, NU, STOK)
    outs = outs.transpose(0, 4, 2, 1, 5, 3)  # -> [core, u, half, strip, c, e]
    logits = outs.reshape(B, A, NA) + np.asarray(dec_b2, np.float32)[None, None, :]
    if _trace:
        return logits.astype(np.float32), res
    return logits.astype(np.float32)


# revision 40
# speedup vs baseline: 1.0079x; 1.0012x over previous
"""CommNet actor kernel for Trainium2 (Bass/Tile), 8-core data-parallel.

Math (per sample, A=32 agents, D=128 obs, H=64 hidden, NA=16 actions):
    h   = tanh(obs @ enc_w + enc_b)
    2 rounds of:  messages = h @ comm_w + comm_b
                  received = (sum_agents(messages) - messages) / (A-1)
                  h = tanh([h, received] @ upd_w + upd_b)
    out = tanh(h @ dec_w1 + dec_b1) @ dec_w2 + dec_b2

The round is folded on the host into  h' = tanh(h @ W1 + s @ W2 + b)  where
s = sum_agents(h), W1 = U_top - comm_w @ U_bot / (A-1), W2 = comm_w @ U_bot / (A-1),
b = comm_b @ U_bot + upd_b   (U_top/U_bot = upd_w[:H], upd_w[H:]).

Device layout: feature-major activations [feat, tok]. Each "unit" is 2048
tokens; the first 1024 tokens (T0) live on SBUF/PSUM partitions 0:64, the
second 1024 (T1) on partitions 64:128. All matmuls run in bf16 (1 cycle/col
moving operand; fp32/f32r move at 2 cyc/col). The 64x64 round weights are
packed as block-diagonal kron(I2, W) so one full-array matmul covers both
token halves. The encoder (128-contraction, 64 out) uses two concurrent
tile_position col-tiles, (0,0) for T0 and (0,64) for T1, so the two halves
stream through the PE at the same time. dec2 (32 out rows) uses four
concurrent 32-partition col-tiles, which also spreads its output over all
128 partitions so the PSUM->SBUF drain copy runs full-lane on the DVE.
tanh/reduce process both halves in single [128, 1024] instructions.

obs is pre-transposed on the host into the exact feature-major DMA layout, so
all HBM traffic is contiguous; the output is stored bf16 in DMA walk order and
decoded (plus the final fp32 dec_b2 add) on the host.
"""

import numpy as np
from contextlib import ExitStack

import concourse.bass as bass
import concourse.bacc as bacc
import concourse.tile as tile
from concourse import mybir
from concourse.bass_utils import run_bass_kernel_spmd

# Problem constants
B, A, D, H, NA = 16384, 32, 128, 64, 16
R = 2
NCORES = 8
S_CORE = B // NCORES          # 2048 samples per core
TOK = S_CORE * A              # 65536 tokens per core
HALF_TOK = 1024               # tokens per half-unit (32 samples)
UNIT_TOK = 2 * HALF_TOK       # 2048 tokens per unit
NU = TOK // UNIT_TOK          # 32 units per core
SAMP_HALF = HALF_TOK // A     # 32 samples per half-unit
STRIP = 4                     # dec2 col-tiles
STOK = HALF_TOK // STRIP      # 256 tokens per dec2 strip
FP = mybir.dt.float32
FR = mybir.dt.float32r
BF = mybir.dt.bfloat16
TANH = mybir.ActivationFunctionType.Tanh


def _f(ap):
    return ap.bitcast(FP)


# wpack16 (bf16) column layout: all matmul weights
_C_ENC = 0                    # enc_w                 [128, 64]
_C_W1 = (64, 192)             # kron(I2, W1_r)        [128, 128] per round
_C_W2 = (320, 448)            # kron(I2, W2_r)        [128, 128] per round
_C_D1 = 576                   # kron(I2, dec_w1)      [128, 128]
_C_D2 = 704                   # kron(I2, dec_w2)      [128, 32]
NW16 = 736
# wpack (fp32) column layout: bias columns enc, r0, r1, dec1 (stacked [b; b])
NW = 4


def build_body(ctx, tc, obs_t, wpack, out, n_units):
    nc = tc.nc
    wpool = ctx.enter_context(tc.tile_pool(name="w", bufs=1))
    obs_pool = ctx.enter_context(tc.tile_pool(name="obs", bufs=12))
    h_pool = ctx.enter_context(tc.tile_pool(name="h", bufs=24))
    s_pool = ctx.enter_context(tc.tile_pool(name="s", bufs=16))
    osb_pool = ctx.enter_context(tc.tile_pool(name="osb", bufs=3))
    ps_pool = ctx.enter_context(tc.tile_pool(name="ps", bufs=3, space="PSUM"))
    po_pool = ctx.enter_context(tc.tile_pool(name="po", bufs=2, space="PSUM"))

    wpack, wpack16 = wpack
    w = wpool.tile([D, NW], FR)
    w16 = wpool.tile([D, NW16], BF)

    def emit_wload():
        nc.sync.dma_start(out=w16[:], in_=wpack16)
        nc.sync.dma_start(out=w[:], in_=wpack)

    def emit_warm():
        # Prime the ACT table (~2.7us TABLE_LOAD+DRAIN) during the DMA ramp so
        # the first real tanh doesn't pay it: a 1-element dummy with no DMA
        # deps. Emitted after the first obs loads — DMA issue shares the
        # Scalar sequencer, so putting this first would delay those issues.
        warm = wpool.tile([128, 1], FP)
        nc.vector.memset(warm[:], 0.0)
        nc.scalar.activation(warm[:], warm[:], TANH)
        # Un-throttle the PE during the DMA ramp: the HAM clock gate keeps the
        # array at 1.2 GHz until it has been busy for a full ~3.4us window, so
        # feed it garbage matmuls (into a scratch psum tile) before the first
        # obs tile lands. Otherwise groups 0-1 run their matmuls at half clock.
        wmm = wpool.tile([128, 512], BF)
        nc.vector.memset(wmm[:], 0.0)
        ps_w = po_pool.tile([128, STOK], FP, tag="po")
        for _ in range(16):
            nc.tensor.matmul(ps_w[0:32, :], lhsT=wmm[:, 0:32],
                             rhs=wmm[:, 0:STOK], tile_position=(0, 0),
                             skip_group_check=True)

    w_enc = w16[:, _C_ENC : _C_ENC + 64]
    w1 = [w16[:, _C_W1[r] : _C_W1[r] + 128] for r in range(R)]
    w2 = [w16[:, _C_W2[r] : _C_W2[r] + 128] for r in range(R)]
    w_d1 = w16[:, _C_D1 : _C_D1 + 128]
    w_d2 = w16[:, _C_D2 : _C_D2 + 32]
    b_enc = _f(w[:, 0:1])
    b_r = [_f(w[:, 1 + r : 2 + r]) for r in range(R)]
    b_d1 = _f(w[:, 3:4])

    c0 = slice(0, 512)
    c1 = slice(512, 1024)

    def emit_load(gi, k):
        # one dma_start per unit (4KB contiguous per partition line): bigger
        # per-group transfers would amortize descriptor overhead better, but
        # tile-granular deps then stall the first unit on the whole transfer
        ob = obs_pool.tile([D, 2, HALF_TOK], BF, tag="obs")
        nc.sync.dma_start(out=ob[:], in_=obs_t[gi, :, k])
        return ob[:, 0, :], ob[:, 1, :]

    def emit_enc_mms(obs0, obs1):
        # two concurrent col-tiles: T0 -> psum partitions 0:64 via array cols
        # 0:63, T1 -> partitions 64:128 via cols 64:127
        ps_e = ps_pool.tile([128, HALF_TOK], FP, tag="ps")
        for cs in (c0, c1):
            nc.tensor.matmul(ps_e[0:64, cs], lhsT=w_enc, rhs=obs0[:, cs],
                             tile_position=(0, 0), skip_group_check=True)
            nc.tensor.matmul(ps_e[64:128, cs], lhsT=w_enc, rhs=obs1[:, cs],
                             tile_position=(0, 64), skip_group_check=True)
        return ps_e

    def emit_tanh(ps, bias, split=False):
        hh = h_pool.tile([128, HALF_TOK], BF, tag="h")
        if split:  # first unit: let the first half-tile tanh start sooner
            nc.scalar.activation(hh[:, c0], ps[:, c0], TANH, bias=bias)
            nc.scalar.activation(hh[:, c1], ps[:, c1], TANH, bias=bias)
        else:
            nc.scalar.activation(hh[:], ps[:], TANH, bias=bias)
        return hh

    def emit_reduce(hh):
        s = s_pool.tile([128, SAMP_HALF], hh.dtype, tag="s")
        with nc.allow_low_precision(
            reason="bf16 agent-sum feeds a bf16 matmul; fp32 internal accum"
        ):
            nc.vector.reduce_sum(
                out=s[:],
                in_=hh.rearrange("p (g a) -> p g a", a=A),
                axis=mybir.AxisListType.X,
            )
        return s

    def emit_round_mms(r, hh, s):
        ns = SAMP_HALF // 2  # samples per 512-token column block
        ps_r = ps_pool.tile([128, HALF_TOK], FP, tag="ps")
        for cs in (c0, c1):
            nc.tensor.matmul(ps_r[:, cs], lhsT=w1[r], rhs=hh[:, cs],
                             start=True, stop=False, skip_group_check=True)
        for b, cs in ((0, c0), (1, c1)):
            sb = s[:, b * ns : (b + 1) * ns].unsqueeze(2).broadcast_to(
                [128, ns, A]
            )
            nc.tensor.matmul(ps_r[:, cs], lhsT=w2[r], rhs=sb,
                             start=False, stop=True, skip_group_check=True)
        return ps_r

    def emit_dec1_mms(hh):
        ps_d = ps_pool.tile([128, HALF_TOK], FP, tag="ps")
        for cs in (c0, c1):
            nc.tensor.matmul(ps_d[:, cs], lhsT=w_d1, rhs=hh[:, cs],
                             skip_group_check=True)
        return ps_d

    def emit_dec2_mms(pre):
        # four concurrent 32-partition col-tiles; strip k covers token block
        # k*256:(k+1)*256 and lands on psum partitions 32k:32k+32
        po = po_pool.tile([128, STOK], FP, tag="po")
        for k in range(STRIP):
            nc.tensor.matmul(po[32 * k : 32 * k + 32, :], lhsT=w_d2,
                             rhs=pre[:, k * STOK : (k + 1) * STOK],
                             tile_position=(0, 32 * k), skip_group_check=True)
        return po

    def emit_dec2_cast(osb, k, po):
        nc.vector.tensor_copy(osb[:, k, :], po[:])

    # Units are emitted in interleaved groups of four (software pipelining
    # at the emission level): one unit's matmuls fill the PE gaps left by
    # another unit's tanh/reduce stages. Without this the PE idles ~1.3us at
    # every stage boundary and the HAM clock-gate re-throttles it to 1.2 GHz.
    groups = []
    u0 = 0
    while u0 < n_units:
        g = 4 if n_units - u0 >= 4 else n_units - u0
        groups.append(list(range(u0, u0 + g)))
        u0 += g
    for gi, grp in enumerate(groups):
        if gi == 0:
            obs = [emit_load(gi, 0)]
            emit_wload()
            obs += [emit_load(gi, k) for k in range(1, len(grp))]
            emit_warm()
        else:
            obs = [emit_load(gi, k) for k in range(len(grp))]
        ps = [emit_enc_mms(*o) for o in obs]
        hs = [emit_tanh(p, b_enc) for p in ps]
        for r in range(R):
            ss = [emit_reduce(hh) for hh in hs]
            ps = [emit_round_mms(r, hh, s) for hh, s in zip(hs, ss)]
            hs = [emit_tanh(p, b_r[r]) for p in ps]
        ps = [emit_dec1_mms(hh) for hh in hs]
        osb = osb_pool.tile([128, len(grp), STOK], BF, tag="osb")
        for k, p in enumerate(ps):
            pre = emit_tanh(p, b_d1)
            po = emit_dec2_mms(pre)
            emit_dec2_cast(osb, k, po)
            if k % 2 == 1:  # flush per unit-pair so the tail drain is short
                nc.sync.dma_start(
                    out=out[:, grp[k - 1] : grp[k] + 1, :],
                    in_=osb[:, k - 1 : k + 1, :],
                )
        if len(grp) % 2 == 1:
            k = len(grp) - 1
            nc.sync.dma_start(
                out=out[:, grp[k] : grp[k] + 1, :], in_=osb[:, k : k + 1, :]
            )


def build_nc(n_units=NU):
    nc = bacc.Bacc(None, target_bir_lowering=False, debug=False)
    n_groups = (n_units + 3) // 4
    obs_t = nc.declare_dram_parameter(
        "obs_t", [n_groups, D, 4, 2, HALF_TOK], BF, isOutput=False
    )
    wpack = nc.declare_dram_parameter("wpack", [D, NW], FR, isOutput=False)
    wpack16 = nc.declare_dram_parameter("wpack16", [D, NW16], BF, isOutput=False)
    out = nc.declare_dram_parameter(
        "out", [128, n_units, STOK], BF, isOutput=True
    )
    with tile.TileContext(nc) as tc:
        with ExitStack() as ctx:
            build_body(ctx, tc, obs_t[:], (wpack[:], wpack16[:]), out[:], n_units)
    nc.compile()
    return nc


def fold_weights(enc_w, enc_b, comm_w, comm_b, upd_w, upd_b, dec_w1, dec_b1, dec_w2):
    """Host-side algebraic fold + packing into the wpack tensors (float64 math)."""
    import ml_dtypes

    f8 = np.float64
    denom = f8(max(A - 1, 1))
    wpack = np.zeros((D, NW), np.float32)
    wpack16 = np.zeros((D, NW16), np.float32)

    def bd(Wm):  # kron(I2, W) for [64, x] -> [128, 2x]
        Wm = np.asarray(Wm, np.float32)
        k, m = Wm.shape
        o = np.zeros((2 * k, 2 * m), np.float32)
        o[:k, :m] = Wm
        o[k:, m:] = Wm
        return o

    wpack16[:, _C_ENC : _C_ENC + 64] = np.asarray(enc_w, np.float32)
    for r in range(R):
        C = np.asarray(comm_w[r], f8)
        Ut = np.asarray(upd_w[r][:H], f8)
        Ub = np.asarray(upd_w[r][H:], f8)
        G = C @ Ub / denom
        W1 = (Ut - G).astype(np.float32)
        W2 = G.astype(np.float32)
        br = (np.asarray(comm_b[r], f8) @ Ub + np.asarray(upd_b[r], f8)).astype(
            np.float32
        )
        wpack16[:, _C_W1[r] : _C_W1[r] + 128] = bd(W1)
        wpack16[:, _C_W2[r] : _C_W2[r] + 128] = bd(W2)
        wpack[0:64, 1 + r] = br
        wpack[64:128, 1 + r] = br
    wpack16[:, _C_D1 : _C_D1 + 128] = bd(dec_w1)
    wpack16[:, _C_D2 : _C_D2 + 32] = bd(dec_w2)
    be = np.asarray(enc_b, np.float32)
    wpack[0:64, 0] = be
    wpack[64:128, 0] = be
    bd1 = np.asarray(dec_b1, np.float32)
    wpack[0:64, 3] = bd1
    wpack[64:128, 3] = bd1
    return wpack, wpack16.astype(ml_dtypes.bfloat16)


def prep_obs(obs):
    """[B, A, D] -> [NCORES, NU//4, D, 4, 2, HALF_TOK] feature-major bf16."""
    import ml_dtypes

    obs6 = np.asarray(obs, np.float32).reshape(NCORES, NU // 4, 4, 2, HALF_TOK, D)
    return np.ascontiguousarray(
        obs6.transpose(0, 1, 5, 2, 3, 4).astype(ml_dtypes.bfloat16)
    )


_NC_CACHE = {}


def _get_nc(n_units=NU):
    if n_units not in _NC_CACHE:
        _NC_CACHE[n_units] = build_nc(n_units)
    return _NC_CACHE[n_units]


def kernel(
    obs,
    enc_w,
    enc_b,
    comm_w,
    comm_b,
    upd_w,
    upd_b,
    dec_w1,
    dec_b1,
    dec_w2,
    dec_b2,
    _trace=False,
    _trace_kwargs=None,
):
    wpack, wpack16 = fold_weights(
        enc_w, enc_b, comm_w, comm_b, upd_w, upd_b, dec_w1, dec_b1, dec_w2
    )
    obs_t = prep_obs(obs)
    nc = _get_nc()
    in_maps = [
        {"obs_t": obs_t[i], "wpack": wpack, "wpack16": wpack16}
        for i in range(NCORES)
    ]
    res = run_bass_kernel_spmd(
        nc,
        in_maps,
        core_ids=list(range(NCORES)),
        trace=_trace,
        **(_trace_kwargs or {}),
    )
    outs = np.stack([res.results[i]["out"] for i in range(NCORES)])
    # device order: out[32*strip + 16*half + e, u, c]; token within the half
    # is strip*256 + c
    outs = np.asarray(outs, np.float32).reshape(NCORES, STRIP, 2, NA, NU, STOK)
    outs = outs.transpose(0, 4, 2, 1, 5, 3)  # -> [core, u, half, strip, c, e]
    logits = outs.reshape(B, A, NA) + np.asarray(dec_b2, np.float32)[None, None, :]
    if _trace:
        return logits.astype(np.float32), res
    return logits.astype(np.float32)
